# revision 1
# baseline (speedup 1.0000x reference)
"""LoRA attention processor on 8 NeuronCores (Trainium2, Bass/Tile).

Reference computation (B=2, S=4096, D=1280, H=8 heads, dh=160, rank-4 LoRA
on K/V):
    q = x @ Wq; k = x @ Wk; v = x @ Wv
    k += (k @ Ak) @ Bk; v += (v @ Av) @ Bv        (LoRA, rank 4)
    attn = softmax(q k^T / sqrt(dh)) v   per head
    out = attn @ Wout + b_out

Sharding: core c handles batch b = c//4 and head pair p = c%4 (columns
320p:320p+320 of the QKV projections, rows of Wout). The LoRA update is
folded into the weights on the host: k + (k@Ak)@Bk == x @ (Wk + Wk@Ak@Bk),
so each core only needs its 320-column slice of the effective weights.
Each core returns a partial output (its heads' contribution to attn@Wout);
the host sums the 4 partials per batch and adds the bias.

On-core layout: scores are computed transposed ([k-pos partitions, q-pos
free]) so softmax's exp runs on ACT over PSUM directly and the PV matmul
needs no transposes: outT[d, q] = sum_j V[j, d] * expT[j, q]. The softmax
denominator rides along as a ones-column appended to V (row 160 of the PV
output), and normalization is applied to outT (160 x 4096 per head)
instead of to the 4096 x 4096 probability matrix. No row-max subtraction:
scores are ~N(0,1) here (|s| < ~7), exp cannot overflow fp32.

All big matmuls run in float32r (TF32-style reduced-precision fp32, full
PE rate at free-dim >= 256 vs 4x slower for exact fp32).
"""

import numpy as np
import ml_dtypes
from contextlib import ExitStack

import concourse.bass as bass
import concourse.tile as tile
from concourse import bacc, mybir
from concourse.bass_utils import run_bass_kernel_spmd

B, S, D = 2, 4096, 1280
H, DH = 8, 160
HP = 320           # head-pair columns per core (2 heads)
N_CORES = 8
SC = 512           # free-dim chunk (q columns / s columns)
NSC = S // SC      # 8
CK = 128           # contraction chunk
NCK = D // CK      # 10
F32 = mybir.dt.float32
F32R = mybir.dt.float32r
BF16 = mybir.dt.bfloat16

_CACHE = {}


def build():
    nc = bacc.Bacc("TRN2", target_bir_lowering=False, debug=False,
                   num_devices=N_CORES)
    # inputs (float32r decl == fp32 bits; PE reads reduced precision)
    xT = nc.dram_tensor("xT", [D, S], F32R, kind="ExternalInput").ap()
    wq = nc.dram_tensor("wq", [D, HP], F32R, kind="ExternalInput").ap()
    wk = nc.dram_tensor("wk", [D, HP], F32R, kind="ExternalInput").ap()
    wv = nc.dram_tensor("wv", [D, HP], F32R, kind="ExternalInput").ap()
    wo = nc.dram_tensor("wo", [HP, D], F32R, kind="ExternalInput").ap()
    ones2 = nc.dram_tensor("ones2", [1, 128], F32, kind="ExternalInput").ap()
    # [...,0]=1 feeds the denominator row of the PV matmul; [...,1]=0 pads
    # the V free dim to an even size (fp32r layout rule)
    onesv = nc.dram_tensor("onesv", [128, 32, 2], F32R, kind="ExternalInput").ap()
    out = nc.dram_tensor("out", [S, D], F32, kind="ExternalOutput").ap()
    # scratch
    qT_d = nc.dram_tensor("qT_d", [HP, S], F32R).ap()
    oT_d = nc.dram_tensor("oT_d", [HP, S], F32R).ap()

    # per-head row chunks of the 320-wide slice: (offset, size)
    hchunks = [[(0, 128), (128, 32)], [(160, 128), (288, 32)]]

    with tile.TileContext(nc) as tc, ExitStack() as top:
        kt_pool = top.enter_context(tc.tile_pool(name="kt", bufs=1))
        v_pool = top.enter_context(tc.tile_pool(name="vp", bufs=1))
        KT = [kt_pool.tile([sz, S], F32R, name=f"KT{i}", tag=f"KT{i}")
              for i, (_, sz) in enumerate(hchunks[0] + hchunks[1])]
        V = [v_pool.tile([128, 32, 162], F32R, name=f"V{h}", tag=f"V{h}")
             for h in range(2)]

        # ---- phase 1: projections QT/KT (transposed) + V (natural) ----
        with ExitStack() as ph1:
            xp = ph1.enter_context(tc.tile_pool(name="xp", bufs=2))
            wp = ph1.enter_context(tc.tile_pool(name="wp", bufs=1))
            pp = ph1.enter_context(tc.tile_pool(name="pp", bufs=4, space="PSUM"))
            sp = ph1.enter_context(tc.tile_pool(name="sp", bufs=3))

            warm = sp.tile([1, 2], F32, tag="warm")
            nc.vector.memset(warm[:], 0.0)
            warm2 = sp.tile([1, 2], F32, tag="warm2")
            nc.scalar.activation(warm2[:], warm[:],
                                 mybir.ActivationFunctionType.Exp)
            wts = {}
            for nm, src in (("wq", wq), ("wk", wk), ("wv", wv)):
                for c in range(NCK):
                    t = wp.tile([CK, HP], F32R, name=f"{nm}_{c}", tag=f"{nm}_{c}")
                    nc.sync.dma_start(t[:], src[c * CK:(c + 1) * CK, :])
                    wts[(nm, c)] = t
            for h in range(2):
                nc.sync.dma_start(V[h][:, :, 160:162], onesv[:])

            for sc in range(NSC):
                xts = []
                for c in range(NCK):
                    xt = xp.tile([CK, SC], F32R, tag=f"xt{c}")
                    nc.sync.dma_start(xt[:], xT[c * CK:(c + 1) * CK,
                                                 sc * SC:(sc + 1) * SC])
                    xts.append(xt)
                # QT / KT chunks: psum[m, q] = sum_c w[c, m].T @ xT[c, q]
                for nm, dst in (("wq", None), ("wk", KT)):
                    for i, (off, msz) in enumerate(hchunks[0] + hchunks[1]):
                        ps = pp.tile([msz, SC], F32, tag="ps")
                        for c in range(NCK):
                            nc.tensor.matmul(
                                ps[:], wts[(nm, c)][:, off:off + msz], xts[c][:],
                                start=(c == 0), stop=(c == NCK - 1))
                        st = sp.tile([msz, SC], F32R, tag=f"st{msz}")
                        nc.vector.tensor_copy(st[:], ps[:])
                        if dst is None:
                            nc.sync.dma_start(
                                qT_d[off:off + msz, sc * SC:(sc + 1) * SC], st[:])
                        else:
                            nc.vector.tensor_copy(
                                dst[i][:, sc * SC:(sc + 1) * SC], ps[:])
                # V natural: psum[s, dv] = xT[c, s].T @ wv[c, :]
                for st4 in range(4):
                    s0 = sc * 4 + st4
                    ps = pp.tile([128, HP], F32, tag="psv")
                    for c in range(NCK):
                        nc.tensor.matmul(
                            ps[:], xts[c][:, st4 * 128:(st4 + 1) * 128],
                            wts[("wv", c)][:], start=(c == 0), stop=(c == NCK - 1))
                    for h in range(2):
                        nc.vector.tensor_copy(V[h][:, s0, 0:160],
                                              ps[:, h * 160:(h + 1) * 160])

        # ---- phase 2: attention per head ----
        with ExitStack() as ph2:
            qp = ph2.enter_context(tc.tile_pool(name="qp", bufs=2))
            scp = ph2.enter_context(tc.tile_pool(name="scp", bufs=3, space="PSUM"))
            ovp = ph2.enter_context(tc.tile_pool(name="ovp", bufs=2, space="PSUM"))
            rbp = ph2.enter_context(tc.tile_pool(name="rbp", bufs=1, space="PSUM"))
            ep = ph2.enter_context(tc.tile_pool(name="ep", bufs=3))
            np_ = ph2.enter_context(tc.tile_pool(name="np", bufs=2))
            o2 = ph2.enter_context(tc.tile_pool(name="o2", bufs=1))
            ones2_t = o2.tile([1, 128], F32)
            nc.sync.dma_start(ones2_t[:], ones2[:])

            qts = {}
            for h in range(2):
                (offA, _), (offB, _) = hchunks[h]
                for qc in range(NSC):
                    qs = slice(qc * SC, (qc + 1) * SC)
                    qA = qp.tile([128, SC], F32R, tag=f"qA{h}_{qc}", bufs=1)
                    qB = qp.tile([32, SC], F32R, tag=f"qB{h}_{qc}", bufs=1)
                    nc.sync.dma_start(qA[:], qT_d[offA:offA + 128, qs])
                    nc.sync.dma_start(qB[:], qT_d[offB:offB + 32, qs])
                    qts[(h, qc)] = (qA, qB)

            for h in range(2):
                (offA, _), (offB, _) = hchunks[h]
                ktA, ktB = KT[2 * h], KT[2 * h + 1]
                for qc in range(NSC):
                    qs = slice(qc * SC, (qc + 1) * SC)
                    qA, qB = qts[(h, qc)]
                    oA = ovp.tile([128, SC], F32, tag="oA")
                    oB = ovp.tile([34, SC], F32, tag="oB")
                    for j in range(32):
                        js = slice(j * 128, (j + 1) * 128)
                        sc_ps = scp.tile([128, SC], F32, tag="sc")
                        nc.tensor.matmul(sc_ps[:], ktA[:, js], qA[:],
                                         start=True, stop=False)
                        nc.tensor.matmul(sc_ps[:], ktB[:, js], qB[:],
                                         start=False, stop=True)
                        ex = ep.tile([128, SC], F32R, tag="ex")
                        nc.scalar.activation(ex[:], sc_ps[:],
                                             mybir.ActivationFunctionType.Exp)
                        nc.tensor.matmul(oA[:], V[h][:, j, 0:128], ex[:],
                                         start=(j == 0), stop=(j == 31))
                        nc.tensor.matmul(oB[:], V[h][:, j, 128:162], ex[:],
                                         start=(j == 0), stop=(j == 31))
                    # normalize by denominator (row 32 of oB) and store
                    rec = np_.tile([1, SC], F32, tag="rec")
                    nc.vector.reciprocal(rec[:], oB[32:33, :])
                    rb = rbp.tile([128, SC], F32, tag="rb")
                    nc.tensor.matmul(rb[:], ones2_t[:], rec[:],
                                     start=True, stop=True)
                    rbs = np_.tile([128, SC], F32, tag="rbs")
                    nc.vector.tensor_copy(rbs[:], rb[:])
                    onA = np_.tile([128, SC], F32R, tag="onA")
                    onB = np_.tile([32, SC], F32R, tag="onB")
                    nc.vector.tensor_mul(onA[:], oA[:], rbs[:])
                    nc.vector.tensor_mul(onB[:], oB[0:32, :], rbs[0:32, :])
                    nc.sync.dma_start(oT_d[offA:offA + 128, qs], onA[:])
                    nc.sync.dma_start(oT_d[offB:offB + 32, qs], onB[:])

        # ---- phase 3: output projection (partial over this core's cols) ----
        with ExitStack() as ph3:
            op = ph3.enter_context(tc.tile_pool(name="op", bufs=1))
            wop = ph3.enter_context(tc.tile_pool(name="wop", bufs=1))
            fp = ph3.enter_context(tc.tile_pool(name="fp", bufs=4, space="PSUM"))
            fs = ph3.enter_context(tc.tile_pool(name="fs", bufs=3))
            chunks = hchunks[0] + hchunks[1]
            woc = []
            for i, (off, msz) in enumerate(chunks):
                w = wop.tile([msz, D], F32R, name=f"wo{i}", tag=f"wo{i}")
                nc.sync.dma_start(w[:], wo[off:off + msz, :])
                woc.append(w)
            for qc in range(NSC):
                qs = slice(qc * SC, (qc + 1) * SC)
                oTc = []
                for i, (off, msz) in enumerate(chunks):
                    t = op.tile([msz, SC], F32R, tag=f"oT{i}", bufs=2)
                    nc.sync.dma_start(t[:], oT_d[off:off + msz, qs])
                    oTc.append(t)
                for st4 in range(4):
                    ss = slice(st4 * 128, (st4 + 1) * 128)
                    row = qc * SC + st4 * 128
                    ot = fs.tile([128, D], F32, tag="ot")
                    for oc, osz in ((0, 512), (512, 512), (1024, 256)):
                        ps = fp.tile([128, osz], F32, tag=f"fo{osz}")
                        for i in range(4):
                            nc.tensor.matmul(ps[:], oTc[i][:, ss],
                                             woc[i][:, oc:oc + osz],
                                             start=(i == 0), stop=(i == 3))
                        nc.vector.tensor_copy(ot[:, oc:oc + osz], ps[:])
                    nc.sync.dma_start(out[row:row + 128, :], ot[:])

    nc.compile()
    return nc


def kernel(hidden_states, w_q, w_k, w_v, lora_k_a, lora_k_b,
           lora_v_a, lora_v_b, w_out, b_out):
    f64 = np.float64
    wk_eff = (w_k.astype(f64)
              + w_k.astype(f64) @ lora_k_a.astype(f64) @ lora_k_b.astype(f64)
              ).astype(np.float32)
    wv_eff = (w_v.astype(f64)
              + w_v.astype(f64) @ lora_v_a.astype(f64) @ lora_v_b.astype(f64)
              ).astype(np.float32)
    wq_s = (w_q.astype(f64) / np.sqrt(DH)).astype(np.float32)

    ones2 = np.ones((1, 128), np.float32)
    onesv = np.zeros((128, 32, 2), np.float32)
    onesv[:, :, 0] = 1.0
    xT = [np.ascontiguousarray(np.asarray(hidden_states)[b].T) for b in range(B)]

    in_maps = []
    for c in range(N_CORES):
        b, p = c // 4, c % 4
        cols = slice(p * HP, (p + 1) * HP)
        in_maps.append({
            "xT": xT[b],
            "wq": np.ascontiguousarray(wq_s[:, cols]),
            "wk": np.ascontiguousarray(wk_eff[:, cols]),
            "wv": np.ascontiguousarray(wv_eff[:, cols]),
            "wo": np.ascontiguousarray(w_out[cols, :]),
            "ones2": ones2,
            "onesv": onesv,
        })

    global _last_in_maps
    _last_in_maps = in_maps
    if "nc" not in _CACHE:
        _CACHE["nc"] = build()
    res = run_bass_kernel_spmd(_CACHE["nc"], in_maps, list(range(N_CORES)))

    out = np.zeros((B, S, D), np.float32)
    for c in range(N_CORES):
        out[c // 4] += res.results[c]["out"]
    out += np.asarray(b_out, np.float32)
    return out



# revision 12
# speedup vs baseline: 1.8221x; 1.8221x over previous
"""LoRA attention processor on 8 NeuronCores (Trainium2, Bass/Tile).

Reference computation (B=2, S=4096, D=1280, H=8 heads, dh=160, rank-4 LoRA
on K/V):
    q = x @ Wq; k = x @ Wk; v = x @ Wv
    k += (k @ Ak) @ Bk; v += (v @ Av) @ Bv        (LoRA, rank 4)
    attn = softmax(q k^T / sqrt(dh)) v   per head
    out = attn @ Wout + b_out
LoRA folded into weights host-side: k + (k@Ak)@Bk == x @ (Wk + Wk@Ak@Bk).

Sharding: core c handles batch b = c//4 and head pair p = c%4 (columns
320p:320p+320 of the QKV projections, rows of Wout). Each core returns a
partial output (its heads' contribution to attn@Wout); the host sums the
4 partials per batch and adds the bias.

Within a core's 320-column slice the columns are permuted host-side to
[h0 dims 0:128 | h1 dims 0:128 | h0 dims 128:160 | h1 dims 128:160] so
that per-head work splits into a K=128 "A" chunk and a K=32 "B" chunk,
with the two heads' B chunks stacked on partitions 0:32 / 32:64 of one
tile.  The two B-chunk score matmuls then land on different PE row
groups (tile_position (0,0) / (32,0) auto-derived from base partitions)
and execute concurrently, costing one matmul slot instead of two.

On-core layout: scores are computed transposed ([k-pos partitions, q-pos
free]) so softmax's exp runs on ACT over PSUM directly and the PV matmul
needs no transposes. The softmax denominator rides along as a ones
column appended to each head's V B-chunk. No row-max subtraction: scores
are ~N(0,1) here, exp cannot overflow fp32.

Perf-critical structure (the baseline ran the PE at 1.2 GHz for 86% of
the kernel because each attention step stalled on the exp semaphore and
the HAM clock gate never saw a fully-busy window):
  - the j-loop is software-pipelined: PV matmuls for step j are emitted
    two steps after the score matmuls for j, so the PE never waits on
    the ACT exp and streams back-to-back matmuls,
  - QT and the attention output live in SBUF (no DRAM round trips),
  - softmax normalization (reciprocal + broadcast + scale) is deferred
    into the next q-chunk's matmul stream so the PE-side broadcast
    matmul never waits on the DVE,
  - Q/K are stored bf16 (halves SBUF, same PE rate), V/exp stay fp32r.
"""

import numpy as np
from contextlib import ExitStack

import concourse.bass as bass
import concourse.tile as tile
from concourse import bacc, mybir
from concourse.bass_utils import run_bass_kernel_spmd

B, S, D = 2, 4096, 1280
H, DH = 8, 160
HP = 320           # head-pair columns per core (2 heads)
N_CORES = 8
SC = 512           # free-dim chunk (q columns)
NSC = S // SC      # 8
CK = 128           # contraction chunk
NCK = D // CK      # 10
NJ = S // 128      # 32 key blocks
F32 = mybir.dt.float32
F32R = mybir.dt.float32r
BF16 = mybir.dt.bfloat16

_CACHE = {}


def build():
    nc = bacc.Bacc("TRN2", target_bir_lowering=False, debug=False,
                   num_devices=N_CORES)
    xT = nc.dram_tensor("xT", [D, S], F32R, kind="ExternalInput").ap()
    wq = nc.dram_tensor("wq", [D, HP], F32R, kind="ExternalInput").ap()
    wk = nc.dram_tensor("wk", [D, HP], F32R, kind="ExternalInput").ap()
    wv = nc.dram_tensor("wv", [D, HP], F32R, kind="ExternalInput").ap()
    wo = nc.dram_tensor("wo", [HP, D], F32R, kind="ExternalInput").ap()
    ones2 = nc.dram_tensor("ones2", [1, 128], F32, kind="ExternalInput").ap()
    # [...,0]=1 feeds the denominator row of the PV matmul; [...,1]=0 pads
    # the V free dim to an even size (fp32r layout rule)
    onesv = nc.dram_tensor("onesv", [128, NJ, 2], F32R, kind="ExternalInput").ap()
    out = nc.dram_tensor("out", [S, D], F32, kind="ExternalOutput").ap()

    with tile.TileContext(nc) as tc, ExitStack() as top:
        # persistent K/Q (transposed, bf16) and V (natural, fp32r)
        kq_pool = top.enter_context(tc.tile_pool(name="kq", bufs=1))
        KTA = [kq_pool.tile([128, S], BF16, name=f"KTA{h}", tag=f"KTA{h}")
               for h in range(2)]
        KTB = kq_pool.tile([64, S], BF16, name="KTB", tag="KTB")
        QTA = [kq_pool.tile([128, S], BF16, name=f"QTA{h}", tag=f"QTA{h}")
               for h in range(2)]
        QTB = kq_pool.tile([64, S], BF16, name="QTB", tag="QTB")
        VA = [kq_pool.tile([128, NJ, 128], F32R, name=f"VA{h}", tag=f"VA{h}")
              for h in range(2)]
        # per head: 32 v-dims, a ones column (softmax denominator), a pad
        VB = [kq_pool.tile([128, NJ, 34], F32R, name=f"VB{h}", tag=f"VB{h}")
              for h in range(2)]
        o2 = top.enter_context(tc.tile_pool(name="o2", bufs=1))
        ones2_t = o2.tile([1, 128], F32)
        nc.sync.dma_start(ones2_t[:], ones2[:])
        for h in range(2):
            nc.sync.dma_start(VB[h][:, :, 32:34], onesv[:])

        # ---- phase 1: projections QT/KT (transposed) + V (natural) ----
        with ExitStack() as ph1:
            xp = ph1.enter_context(tc.tile_pool(name="xp", bufs=2))
            wp = ph1.enter_context(tc.tile_pool(name="wp", bufs=1))
            ppq = ph1.enter_context(tc.tile_pool(name="ppq", bufs=3, space="PSUM"))
            ppb = ph1.enter_context(tc.tile_pool(name="ppb", bufs=2, space="PSUM"))
            ppv = ph1.enter_context(tc.tile_pool(name="ppv", bufs=3, space="PSUM"))
            sp = ph1.enter_context(tc.tile_pool(name="sp", bufs=1))

            warm = sp.tile([1, 2], F32, tag="warm")
            nc.vector.memset(warm[:], 0.0)
            warm2 = sp.tile([1, 2], F32, tag="warm2")
            nc.scalar.activation(warm2[:], warm[:],
                                 mybir.ActivationFunctionType.Exp)
            wts = {}
            for nm, src in (("wq", wq), ("wk", wk), ("wv", wv)):
                for c in range(NCK):
                    t = wp.tile([CK, HP], F32R, name=f"{nm}_{c}", tag=f"{nm}_{c}")
                    nc.sync.dma_start(t[:], src[c * CK:(c + 1) * CK, :])
                    wts[(nm, c)] = t

            # output row chunks of QT/KT: (dst-kind, col offset, rows)
            for sc in range(NSC):
                cs = slice(sc * SC, (sc + 1) * SC)
                xts = []
                for c in range(NCK):
                    xt = xp.tile([CK, SC], F32R, tag=f"xt{c}")
                    nc.sync.dma_start(xt[:], xT[c * CK:(c + 1) * CK, cs])
                    xts.append(xt)
                for nm, dstA, dstB in (("wq", QTA, QTB), ("wk", KTA, KTB)):
                    for dst, off, msz in ((dstA[0], 0, 128), (dstA[1], 128, 128),
                                          (dstB, 256, 64)):
                        pool = ppq if msz == 128 else ppb
                        ps = pool.tile([msz, SC], F32, tag=f"ps{msz}")
                        for c in range(NCK):
                            nc.tensor.matmul(
                                ps[:], wts[(nm, c)][:, off:off + msz], xts[c][:],
                                start=(c == 0), stop=(c == NCK - 1))
                        nc.vector.tensor_copy(dst[:, cs], ps[:])
                # V natural: psum[s, dv] = xT[c, s].T @ wv[c, :]
                for st4 in range(4):
                    j = sc * 4 + st4
                    ps = ppv.tile([128, HP], F32, tag="psv")
                    for c in range(NCK):
                        nc.tensor.matmul(
                            ps[:], xts[c][:, st4 * 128:(st4 + 1) * 128],
                            wts[("wv", c)][:], start=(c == 0), stop=(c == NCK - 1))
                    for h in range(2):
                        nc.vector.tensor_copy(VA[h][:, j, :],
                                              ps[:, h * 128:(h + 1) * 128])
                        nc.vector.tensor_copy(VB[h][:, j, 0:32],
                                              ps[:, 256 + h * 32:256 + (h + 1) * 32])

        # ---- phase 2+3: attention + output projection ----
        with ExitStack() as ph23:
            wop = ph23.enter_context(tc.tile_pool(name="wop", bufs=1))
            otp = ph23.enter_context(tc.tile_pool(name="otp", bufs=1))
            ep = ph23.enter_context(tc.tile_pool(name="ep", bufs=6))
            np_ = ph23.enter_context(tc.tile_pool(name="np", bufs=2))

            woA = [wop.tile([128, D], F32R, name=f"woA{h}", tag=f"woA{h}")
                   for h in range(2)]
            woB = [wop.tile([32, D], F32R, name=f"woB{h}", tag=f"woB{h}")
                   for h in range(2)]
            for h in range(2):
                nc.sync.dma_start(woA[h][:], wo[h * 128:(h + 1) * 128, :])
                nc.sync.dma_start(woB[h][:], wo[256 + h * 32:256 + (h + 1) * 32, :])
            # per-qc attention output chunks (normalized, transposed)
            oTA = {(h, qc): otp.tile([128, SC], F32R, name=f"oTA{h}_{qc}", tag=f"oTA{h}_{qc}")
                   for h in range(2) for qc in range(NSC)}
            oTB = {(h, qc): otp.tile([32, SC], F32R, name=f"oTB{h}_{qc}", tag=f"oTB{h}_{qc}")
                   for h in range(2) for qc in range(NSC)}

            with ExitStack() as ph2:
                scp = ph2.enter_context(tc.tile_pool(name="scp", bufs=3, space="PSUM"))
                ovp = ph2.enter_context(tc.tile_pool(name="ovp", bufs=1, space="PSUM"))
                obp = ph2.enter_context(tc.tile_pool(name="obp", bufs=1, space="PSUM"))
                rbp = ph2.enter_context(tc.tile_pool(name="rbp", bufs=1, space="PSUM"))

                def emit_norm(qc, oA, oB):
                    """Normalize this qc's PV accumulators into oTA/oTB.
                    PE content: two broadcast matmuls; rest is DVE."""
                    qs = slice(qc * SC, (qc + 1) * SC)
                    for h in range(2):
                        den = np_.tile([1, SC], F32, tag="den")
                        nc.vector.tensor_copy(den[:], oB[h][32:33, :])
                        rec = np_.tile([1, SC], F32, tag="rec")
                        nc.vector.reciprocal_approx_fast(rec[:], den[:])
                        rb = rbp.tile([128, SC], F32, tag="rb")
                        nc.tensor.matmul(rb[:], ones2_t[:], rec[:],
                                         start=True, stop=True)
                        rbs = np_.tile([128, SC], F32, tag="rbs")
                        nc.vector.tensor_copy(rbs[:], rb[:])
                        nc.vector.tensor_mul(oTA[(h, qc)][:], oA[h][:], rbs[:])
                        nc.vector.tensor_mul(oTB[(h, qc)][:], oB[h][0:32, :],
                                             rbs[0:32, :])

                # One flat stream of (qc, j) steps. PV matmuls trail the
                # score matmuls by 2 steps (across qc boundaries) so the PE
                # never waits on the ACT exp; each qc's normalization is
                # emitted as soon as its last PV is, and the sim-driven
                # scheduler slots it where its inputs are ready.
                steps = [(qc, j) for qc in range(NSC) for j in range(NJ)]
                accs, exs = {}, {}

                def emit_scores(idx):
                    qc, j = steps[idx]
                    if j == 0:
                        accs[qc] = (
                            [ovp.tile([128, SC], F32, name=f"oA{h}_{qc}",
                                      tag=f"oA{h}") for h in range(2)],
                            [obp.tile([34, SC], F32, name=f"oB{h}_{qc}",
                                      tag=f"oB{h}") for h in range(2)])
                    qs = slice(qc * SC, (qc + 1) * SC)
                    js = slice(j * 128, (j + 1) * 128)
                    # both heads; B chunks pair up on PE row groups 0/1 and
                    # run concurrently
                    sc_ps = [scp.tile([128, SC], F32, name=f"sc{idx}_{h}",
                                      tag="sc") for h in range(2)]
                    for h in range(2):
                        nc.tensor.matmul(sc_ps[h][:], KTA[h][:, js],
                                         QTA[h][:, qs], start=True, stop=False)
                    for h in range(2):
                        nc.tensor.matmul(sc_ps[h][:], KTB[32 * h:32 * h + 32, js],
                                         QTB[32 * h:32 * h + 32, qs],
                                         start=False, stop=True)
                    for h in range(2):
                        ex = ep.tile([128, SC], F32R, tag="ex")
                        nc.scalar.activation(ex[:], sc_ps[h][:],
                                             mybir.ActivationFunctionType.Exp)
                        exs[(h, idx)] = ex

                def emit_pv(idx):
                    qc, j = steps[idx]
                    oA, oB = accs[qc]
                    for h in range(2):
                        nc.tensor.matmul(oA[h][:], VA[h][:, j, :],
                                         exs[(h, idx)][:],
                                         start=(j == 0), stop=(j == NJ - 1))
                        nc.tensor.matmul(oB[h][:], VB[h][:, j, :],
                                         exs[(h, idx)][:],
                                         start=(j == 0), stop=(j == NJ - 1))
                        del exs[(h, idx)]
                    if j == NJ - 1:
                        emit_norm(qc, oA, oB)
                        del accs[qc]

                for idx in range(len(steps)):
                    emit_scores(idx)
                    if idx >= 2:
                        emit_pv(idx - 2)
                emit_pv(len(steps) - 2)
                emit_pv(len(steps) - 1)

            # ---- phase 3: output projection (partial over this core's cols) ----
            with ExitStack() as ph3:
                fp = ph3.enter_context(tc.tile_pool(name="fp", bufs=4, space="PSUM"))
                fs = ph3.enter_context(tc.tile_pool(name="fs", bufs=2))
                for qc in range(NSC):
                    for st4 in range(4):
                        ss = slice(st4 * 128, (st4 + 1) * 128)
                        row = qc * SC + st4 * 128
                        ot = fs.tile([128, D], F32, tag="ot")
                        for oc, osz in ((0, 512), (512, 512), (1024, 256)):
                            ps = fp.tile([128, osz], F32, tag=f"fo{osz}")
                            k = 0
                            for h in range(2):
                                nc.tensor.matmul(ps[:], oTA[(h, qc)][:, ss],
                                                 woA[h][:, oc:oc + osz],
                                                 start=(k == 0), stop=False)
                                k += 1
                            for h in range(2):
                                nc.tensor.matmul(ps[:], oTB[(h, qc)][:, ss],
                                                 woB[h][:, oc:oc + osz],
                                                 start=False, stop=(k == 3))
                                k += 1
                            nc.vector.tensor_copy(ot[:, oc:oc + osz], ps[:])
                        nc.sync.dma_start(out[row:row + 128, :], ot[:])

    nc.compile()
    return nc


def kernel(hidden_states, w_q, w_k, w_v, lora_k_a, lora_k_b,
           lora_v_a, lora_v_b, w_out, b_out):
    f64 = np.float64
    wk_eff = (w_k.astype(f64)
              + w_k.astype(f64) @ lora_k_a.astype(f64) @ lora_k_b.astype(f64)
              ).astype(np.float32)
    wv_eff = (w_v.astype(f64)
              + w_v.astype(f64) @ lora_v_a.astype(f64) @ lora_v_b.astype(f64)
              ).astype(np.float32)
    wq_s = (w_q.astype(f64) / np.sqrt(DH)).astype(np.float32)

    ones2 = np.ones((1, 128), np.float32)
    onesv = np.zeros((128, NJ, 2), np.float32)
    onesv[:, :, 0] = 1.0
    xT = [np.ascontiguousarray(np.asarray(hidden_states)[b].T) for b in range(B)]
    # within each 320-col head-pair slice: [h0 A | h1 A | h0 B | h1 B]
    perm = np.concatenate([np.arange(0, 128), np.arange(160, 288),
                           np.arange(128, 160), np.arange(288, 320)])

    in_maps = []
    for c in range(N_CORES):
        b, p = c // 4, c % 4
        cols = p * HP + perm
        in_maps.append({
            "xT": xT[b],
            "wq": np.ascontiguousarray(wq_s[:, cols]),
            "wk": np.ascontiguousarray(wk_eff[:, cols]),
            "wv": np.ascontiguousarray(wv_eff[:, cols]),
            "wo": np.ascontiguousarray(w_out[cols, :]),
            "ones2": ones2,
            "onesv": onesv,
        })

    global _last_in_maps
    _last_in_maps = in_maps
    if "nc" not in _CACHE:
        _CACHE["nc"] = build()
    res = run_bass_kernel_spmd(_CACHE["nc"], in_maps, list(range(N_CORES)))

    out = np.zeros((B, S, D), np.float32)
    for c in range(N_CORES):
        out[c // 4] += res.results[c]["out"]
    out += np.asarray(b_out, np.float32)
    return out


# revision 25
# speedup vs baseline: 2.1596x; 1.1853x over previous
"""LoRA attention processor on 8 NeuronCores (Trainium2, Bass/Tile).

Reference computation (B=2, S=4096, D=1280, H=8 heads, dh=160, rank-4 LoRA
on K/V):
    q = x @ Wq; k = x @ Wk; v = x @ Wv
    k += (k @ Ak) @ Bk; v += (v @ Av) @ Bv        (LoRA, rank 4)
    attn = softmax(q k^T / sqrt(dh)) v   per head
    out = attn @ Wout + b_out
LoRA folded into weights host-side: k + (k@Ak)@Bk == x @ (Wk + Wk@Ak@Bk).

Sharding: core c handles batch b = c//4 and head pair p = c%4 (columns
320p:320p+320 of the QKV projections, rows of Wout). Each core returns a
partial output (its heads' contribution to attn@Wout); the host sums the
4 partials per batch and adds the bias.

Within a core's 320-column slice the columns are permuted host-side to
[h0 dims 0:128 | h1 dims 0:128 | h0 dims 128:160 | h1 dims 128:160] so
that per-head work splits into a K=128 "A" chunk and a K=32 "B" chunk,
with the two heads' B chunks stacked on partitions 0:32 / 32:64 of one
tile.  The two B-chunk score matmuls then land on different PE row
groups (tile_position (0,0) / (32,0) auto-derived from base partitions)
and execute concurrently, costing one matmul slot instead of two.

On-core layout: scores are computed transposed ([k-pos partitions, q-pos
free]) so softmax's exp runs on ACT over PSUM directly and the PV matmul
needs no transposes. The softmax denominator rides along as a ones
column appended to each head's V B-chunk. No row-max subtraction: scores
are ~N(0,1) here, exp cannot overflow fp32.

Perf-critical structure (the baseline ran the PE at 1.2 GHz for 86% of
the kernel because each attention step stalled on the exp semaphore and
the HAM clock gate never saw a fully-busy window):
  - the j-loop is software-pipelined: PV matmuls for step j are emitted
    two steps after the score matmuls for j, so the PE never waits on
    the ACT exp and streams back-to-back matmuls,
  - QT and the attention output live in SBUF (no DRAM round trips),
  - softmax normalization (reciprocal + broadcast + scale) is deferred
    into the next q-chunk's matmul stream so the PE-side broadcast
    matmul never waits on the DVE,
  - Q/K are stored bf16 (halves SBUF, same PE rate), V/exp stay fp32r.
"""

import numpy as np
import ml_dtypes
from contextlib import ExitStack

import concourse.bass as bass
import concourse.tile as tile
from concourse import bacc, mybir, library_config
from concourse.bass_utils import run_bass_kernel_spmd

B, S, D = 2, 4096, 1280
H, DH = 8, 160
HP = 320           # head-pair columns per core (2 heads)
N_CORES = 8
SC = 512           # free-dim chunk (q columns)
NSC = S // SC      # 8
CK = 128           # contraction chunk
NCK = D // CK      # 10
NJ = S // 128      # 32 key blocks
F32 = mybir.dt.float32
F32R = mybir.dt.float32r
BF16 = mybir.dt.bfloat16

_CACHE = {}


def build():
    nc = bacc.Bacc("TRN2", target_bir_lowering=False, debug=False,
                   num_devices=N_CORES)
    xT = nc.dram_tensor("xT", [D, S], F32R, kind="ExternalInput").ap()
    wq = nc.dram_tensor("wq", [D, HP], F32R, kind="ExternalInput").ap()
    wk = nc.dram_tensor("wk", [D, HP], F32R, kind="ExternalInput").ap()
    wv = nc.dram_tensor("wv", [D, HP], F32R, kind="ExternalInput").ap()
    wo = nc.dram_tensor("wo", [HP, D], BF16, kind="ExternalInput").ap()
    # [...,0]=1 feeds the denominator row of the PV matmul; [...,1:]=0 pads
    # the V B-chunk stationary up to 68 columns so its matmul stays in
    # 128x128 mode (no column-tiling mode switches)
    onesv = nc.dram_tensor("onesv", [128, NJ, 36], F32R, kind="ExternalInput").ap()
    out = nc.dram_tensor("out", [S, D], F32, kind="ExternalOutput").ap()

    with tile.TileContext(nc) as tc, ExitStack() as top:
        # persistent K/Q (transposed, bf16) and V (natural, fp32r)
        kq_pool = top.enter_context(tc.tile_pool(name="kq", bufs=1))
        KTA = [kq_pool.tile([128, S], BF16, name=f"KTA{h}", tag=f"KTA{h}")
               for h in range(2)]
        KTB = kq_pool.tile([64, S], BF16, name="KTB", tag="KTB")
        QTA = [kq_pool.tile([128, S], BF16, name=f"QTA{h}", tag=f"QTA{h}")
               for h in range(2)]
        QTB = kq_pool.tile([64, S], BF16, name="QTB", tag="QTB")
        VA = [kq_pool.tile([128, NJ, 128], F32R, name=f"VA{h}", tag=f"VA{h}")
              for h in range(2)]
        # per head: 32 v-dims, a ones column (softmax denominator), zero pad
        VB = [kq_pool.tile([128, NJ, 68], F32R, name=f"VB{h}", tag=f"VB{h}")
              for h in range(2)]
        nc.gpsimd.load_library(library_config.attn)

        # ---- phase 1: projections QT/KT (transposed) + V (natural) ----
        with ExitStack() as ph1:
            xp = ph1.enter_context(tc.tile_pool(name="xp", bufs=2))
            wp = ph1.enter_context(tc.tile_pool(name="wp", bufs=1))
            ppq = ph1.enter_context(tc.tile_pool(name="ppq", bufs=3, space="PSUM"))
            ppb = ph1.enter_context(tc.tile_pool(name="ppb", bufs=2, space="PSUM"))
            ppv = ph1.enter_context(tc.tile_pool(name="ppv", bufs=3, space="PSUM"))
            sp = ph1.enter_context(tc.tile_pool(name="sp", bufs=1))

            warm = sp.tile([1, 2], F32, tag="warm")
            nc.vector.memset(warm[:], 0.0)
            warm2 = sp.tile([1, 2], F32, tag="warm2")
            nc.scalar.activation(warm2[:], warm[:],
                                 mybir.ActivationFunctionType.Exp)
            # first column block's x chunks before the weights so the first
            # matmul group's inputs land first; onesv (needed only by
            # phase 2) last
            xts0 = []
            for c in range(NCK):
                xt = xp.tile([CK, SC], F32R, name=f"xt0_{c}", tag=f"xt{c}")
                nc.sync.dma_start(xt[:], xT[c * CK:(c + 1) * CK, 0:SC])
                xts0.append(xt)
            wts = {}
            for nm, src in (("wq", wq), ("wk", wk), ("wv", wv)):
                for c in range(NCK):
                    t = wp.tile([CK, HP], F32R, name=f"{nm}_{c}", tag=f"{nm}_{c}")
                    nc.sync.dma_start(t[:], src[c * CK:(c + 1) * CK, :])
                    wts[(nm, c)] = t
            for h in range(2):
                nc.sync.dma_start(VB[h][:, :, 32:68], onesv[:])

            # output row chunks of QT/KT: (dst-kind, col offset, rows)
            for sc in range(NSC):
                cs = slice(sc * SC, (sc + 1) * SC)
                if sc == 0:
                    xts = xts0
                else:
                    xts = []
                    for c in range(NCK):
                        xt = xp.tile([CK, SC], F32R, name=f"xt{sc}_{c}",
                                     tag=f"xt{c}")
                        nc.sync.dma_start(xt[:], xT[c * CK:(c + 1) * CK, cs])
                        xts.append(xt)
                for nm, dstA, dstB in (("wq", QTA, QTB), ("wk", KTA, KTB)):
                    for dst, off, msz in ((dstA[0], 0, 128), (dstA[1], 128, 128),
                                          (dstB, 256, 64)):
                        pool = ppq if msz == 128 else ppb
                        ps = pool.tile([msz, SC], F32, tag=f"ps{msz}")
                        for c in range(NCK):
                            nc.tensor.matmul(
                                ps[:], wts[(nm, c)][:, off:off + msz], xts[c][:],
                                start=(c == 0), stop=(c == NCK - 1))
                        nc.vector.tensor_copy(dst[:, cs], ps[:])
                # V natural: psum[s, dv] = xT[c, s].T @ wv[c, :]
                for st4 in range(4):
                    j = sc * 4 + st4
                    ps = ppv.tile([128, HP], F32, tag="psv")
                    for c in range(NCK):
                        nc.tensor.matmul(
                            ps[:], xts[c][:, st4 * 128:(st4 + 1) * 128],
                            wts[("wv", c)][:], start=(c == 0), stop=(c == NCK - 1))
                    for h in range(2):
                        nc.vector.tensor_copy(VA[h][:, j, :],
                                              ps[:, h * 128:(h + 1) * 128])
                        nc.vector.tensor_copy(VB[h][:, j, 0:32],
                                              ps[:, 256 + h * 32:256 + (h + 1) * 32])

        # ---- phase 2+3: attention + output projection ----
        with ExitStack() as ph23:
            wop = ph23.enter_context(tc.tile_pool(name="wop", bufs=1))
            otp = ph23.enter_context(tc.tile_pool(name="otp", bufs=1))
            ep = ph23.enter_context(tc.tile_pool(name="ep", bufs=8))
            np_ = ph23.enter_context(tc.tile_pool(name="np", bufs=2))

            woA = [wop.tile([128, D], BF16, name=f"woA{h}", tag=f"woA{h}")
                   for h in range(2)]
            woB = [wop.tile([32, D], BF16, name=f"woB{h}", tag=f"woB{h}")
                   for h in range(2)]
            for h in range(2):
                nc.sync.dma_start(woA[h][:], wo[h * 128:(h + 1) * 128, :])
                nc.sync.dma_start(woB[h][:], wo[256 + h * 32:256 + (h + 1) * 32, :])
            # per-qc attention output chunks (normalized, transposed)
            oTA = {(h, qc): otp.tile([128, SC], BF16, name=f"oTA{h}_{qc}", tag=f"oTA{h}_{qc}")
                   for h in range(2) for qc in range(NSC)}
            oTB = {(h, qc): otp.tile([32, SC], BF16, name=f"oTB{h}_{qc}", tag=f"oTB{h}_{qc}")
                   for h in range(2) for qc in range(NSC)}

            with ExitStack() as ph2:
                scp = ph2.enter_context(tc.tile_pool(name="scp", bufs=3, space="PSUM"))
                ovp = ph2.enter_context(tc.tile_pool(name="ovp", bufs=1, space="PSUM"))
                obp = ph2.enter_context(tc.tile_pool(name="obp", bufs=1, space="PSUM"))
                fp = ph2.enter_context(tc.tile_pool(name="fp", bufs=1, space="PSUM"))
                fs = ph2.enter_context(tc.tile_pool(name="fs", bufs=2))

                def emit_norm(qc, oA, oB):
                    """Normalize this qc's PV accumulators into oTA/oTB.
                    DVE + GpSimd only — the PE is not involved."""
                    for h in range(2):
                        den = np_.tile([1, SC], F32, tag="den")
                        nc.vector.tensor_copy(den[:], oB[h][32:33, :])
                        rec = np_.tile([1, SC], F32, tag="rec")
                        nc.vector.reciprocal_approx_fast(rec[:], den[:])
                        rbs = np_.tile([128, SC], F32, tag="rbs")
                        nc.gpsimd.partition_broadcast(rbs[:], rec[:])
                        nc.vector.tensor_mul(oTA[(h, qc)][:], oA[h][:], rbs[:])
                        nc.vector.tensor_mul(oTB[(h, qc)][:], oB[h][0:32, :],
                                             rbs[0:32, :])

                def emit_ph3(qc, st4):
                    """Output projection for one 128-row block, riding the
                    phase-2 matmul stream."""
                    ss = slice(st4 * 128, (st4 + 1) * 128)
                    row = qc * SC + st4 * 128
                    ot = fs.tile([128, D], F32, name=f"ot{qc}_{st4}", tag="ot")
                    for oc, osz in ((0, 512), (512, 512), (1024, 256)):
                        psf = fp.tile([128, 512], F32, name=f"fo{qc}_{st4}_{oc}",
                                      tag="fo")
                        ps = psf[:, 0:osz]
                        k = 0
                        for h in range(2):
                            nc.tensor.matmul(ps[:], oTA[(h, qc)][:, ss],
                                             woA[h][:, oc:oc + osz],
                                             start=(k == 0), stop=False)
                            k += 1
                        for h in range(2):
                            nc.tensor.matmul(ps[:], oTB[(h, qc)][:, ss],
                                             woB[h][:, oc:oc + osz],
                                             start=False, stop=(k == 3))
                            k += 1
                        nc.vector.tensor_copy(ot[:, oc:oc + osz], ps[:])
                    nc.sync.dma_start(out[row:row + 128, :], ot[:])

                # One flat stream of (qc, j) steps. PV matmuls trail the
                # score matmuls by 3 steps (across qc boundaries) so the PE
                # never waits on the ACT exp; each qc's normalization is
                # emitted as soon as its last PV is, and its output
                # projection blocks are spread through the next qc's steps.
                # The sim-driven scheduler slots everything where inputs are
                # ready.
                LAG = 3
                steps = [(qc, j) for qc in range(NSC) for j in range(NJ)]
                accs, exs = {}, {}
                ph3_pending = []

                def emit_scores(idx):
                    qc, j = steps[idx]
                    if j == 0:
                        accs[qc] = (
                            [ovp.tile([128, SC], F32, name=f"oA{h}_{qc}",
                                      tag=f"oA{h}") for h in range(2)],
                            [obp.tile([68, SC], F32, name=f"oB{h}_{qc}",
                                      tag=f"oB{h}") for h in range(2)])
                    qs = slice(qc * SC, (qc + 1) * SC)
                    js = slice(j * 128, (j + 1) * 128)
                    # both heads; B chunks pair up on PE row groups 0/1 and
                    # run concurrently
                    sc_ps = [scp.tile([128, SC], F32, name=f"sc{idx}_{h}",
                                      tag="sc") for h in range(2)]
                    for h in range(2):
                        nc.tensor.matmul(sc_ps[h][:], KTA[h][:, js],
                                         QTA[h][:, qs], start=True, stop=False)
                    for h in range(2):
                        nc.tensor.matmul(sc_ps[h][:], KTB[32 * h:32 * h + 32, js],
                                         QTB[32 * h:32 * h + 32, qs],
                                         start=False, stop=True)
                    for h in range(2):
                        ex = ep.tile([128, SC], F32R, tag="ex")
                        nc.scalar.activation(ex[:], sc_ps[h][:],
                                             mybir.ActivationFunctionType.Exp)
                        exs[(h, idx)] = ex

                def emit_pv(idx):
                    qc, j = steps[idx]
                    oA, oB = accs[qc]
                    for h in range(2):
                        nc.tensor.matmul(oA[h][:], VA[h][:, j, :],
                                         exs[(h, idx)][:],
                                         start=(j == 0), stop=(j == NJ - 1))
                        nc.tensor.matmul(oB[h][:], VB[h][:, j, :],
                                         exs[(h, idx)][:],
                                         start=(j == 0), stop=(j == NJ - 1))
                        del exs[(h, idx)]
                    if j == NJ - 1:
                        emit_norm(qc, oA, oB)
                        del accs[qc]
                        ph3_pending.extend((qc, st4) for st4 in range(4))

                for idx in range(len(steps)):
                    emit_scores(idx)
                    if idx >= LAG:
                        emit_pv(idx - LAG)
                    if ph3_pending and idx % 8 == 6:
                        emit_ph3(*ph3_pending.pop(0))
                for idx in range(len(steps) - LAG, len(steps)):
                    emit_pv(idx)
                for blk in ph3_pending:
                    emit_ph3(*blk)

    nc.compile()
    return nc


def kernel(hidden_states, w_q, w_k, w_v, lora_k_a, lora_k_b,
           lora_v_a, lora_v_b, w_out, b_out):
    f64 = np.float64
    wk_eff = (w_k.astype(f64)
              + w_k.astype(f64) @ lora_k_a.astype(f64) @ lora_k_b.astype(f64)
              ).astype(np.float32)
    wv_eff = (w_v.astype(f64)
              + w_v.astype(f64) @ lora_v_a.astype(f64) @ lora_v_b.astype(f64)
              ).astype(np.float32)
    wq_s = (w_q.astype(f64) / np.sqrt(DH)).astype(np.float32)

    onesv = np.zeros((128, NJ, 36), np.float32)
    onesv[:, :, 0] = 1.0
    xT = [np.ascontiguousarray(np.asarray(hidden_states)[b].T) for b in range(B)]
    # within each 320-col head-pair slice: [h0 A | h1 A | h0 B | h1 B]
    perm = np.concatenate([np.arange(0, 128), np.arange(160, 288),
                           np.arange(128, 160), np.arange(288, 320)])

    in_maps = []
    for c in range(N_CORES):
        b, p = c // 4, c % 4
        cols = p * HP + perm
        in_maps.append({
            "xT": xT[b],
            "wq": np.ascontiguousarray(wq_s[:, cols]),
            "wk": np.ascontiguousarray(wk_eff[:, cols]),
            "wv": np.ascontiguousarray(wv_eff[:, cols]),
            "wo": np.ascontiguousarray(w_out[cols, :]).astype(ml_dtypes.bfloat16),
            "onesv": onesv,
        })

    global _last_in_maps
    _last_in_maps = in_maps
    if "nc" not in _CACHE:
        _CACHE["nc"] = build()
    res = run_bass_kernel_spmd(_CACHE["nc"], in_maps, list(range(N_CORES)))

    out = np.zeros((B, S, D), np.float32)
    for c in range(N_CORES):
        out[c // 4] += res.results[c]["out"]
    out += np.asarray(b_out, np.float32)
    return out
